# revision 2
# baseline (speedup 1.0000x reference)
"""3-layer LSTM decoder + projection + softmax on 8 trn2 NeuronCores. v3.

Like v2 (PE column-group paired matmuls), but fp16 operands (bf16
drifts past the 2e-2 gate over 256 steps) and the per-step h exchange
uses remote_dma_broadcast (direct SBUF->SBUF push to all 8 cores via
the 16 SDMA engines) instead of ncfw AllGather collectives (~8-14us
each, serialized on the collective cores).

Each core broadcasts its transposed h-slices into every core's hrx
buffer at slot <pid>; slots double as matmul contract k-tiles, so data
lands matmul-ready. Arrival is signaled by remote semaphores (each
sender's broadcast adds 2 per receiver; 16 per full round). Receivers
wait_ge on the PE queue before consuming. hrx is triple-buffered: a
sender can only reach step s+3 after all cores passed their step s+2
waits, which (PE in-order) implies step s+1 reads are done -> no WAR
race.

Step s emission: [wait rsem1>=16*cnt1] phase-a matmuls (rk1/x/lat |
proj/k2h1 head) which need only h1(s-1) or older; h1 evac + broadcast;
[wait rsem23>=16*cnt23] phase-b matmuls (rk3/b3/k3h2 | k2h1 tail/
rk2/b2) which need h2(s-2)/h3(s-3... fresh round); h2/h3 evac +
broadcast; softmax/outputs.
"""
import numpy as np

import concourse.bass as bass
import concourse.bacc as bacc
import concourse.mybir as mybir
from concourse.tile import TileContext
from concourse.bass_utils import run_bass_kernel_spmd

F32 = mybir.dt.float32
FP16 = mybir.dt.float16
ACT = mybir.ActivationFunctionType
ALU = mybir.AluOpType

B, T, U, V, L, F = 64, 256, 1024, 512, 512, 64
NCORE = 8
GS = 512          # gates per core (4 x 128 units)
US = 128          # units per core
KT = U // 128     # 8 contract tiles over hidden
TWIN = T // NCORE
SLOT = 3 * B      # per-sender slot in hrx: [h1 | h3 | h2] x B columns


def build(t_steps=T):
    nc = bacc.Bacc("TRN2", target_bir_lowering=False)
    TT = t_steps

    latT = nc.dram_tensor("latT", [L, B], FP16, kind="ExternalInput")
    xT = nc.dram_tensor("xT", [F, TT, B], FP16, kind="ExternalInput")
    k1L = nc.dram_tensor("k1L", [L, GS], FP16, kind="ExternalInput")
    k1x = nc.dram_tensor("k1x", [F, GS], FP16, kind="ExternalInput")
    rk1 = nc.dram_tensor("rk1", [U, GS], FP16, kind="ExternalInput")
    k2 = nc.dram_tensor("k2", [U, GS], FP16, kind="ExternalInput")
    rk2 = nc.dram_tensor("rk2", [U, GS], FP16, kind="ExternalInput")
    k3 = nc.dram_tensor("k3", [U, GS], FP16, kind="ExternalInput")
    rk3 = nc.dram_tensor("rk3", [U, GS], FP16, kind="ExternalInput")
    wp = nc.dram_tensor("wp", [U, V], FP16, kind="ExternalInput")
    b1r = nc.dram_tensor("b1r", [1, GS], FP16, kind="ExternalInput")
    b2r = nc.dram_tensor("b2r", [1, GS], FP16, kind="ExternalInput")
    b3r = nc.dram_tensor("b3r", [1, GS], FP16, kind="ExternalInput")
    bvr = nc.dram_tensor("bvr", [1, V], FP16, kind="ExternalInput")
    eye64 = nc.dram_tensor("eye64", [B, B], FP16, kind="ExternalInput")
    eye128 = nc.dram_tensor("eye128", [128, 128], FP16, kind="ExternalInput")
    ones1 = nc.dram_tensor("ones1", [1, B], FP16, kind="ExternalInput")

    y_out = nc.dram_tensor("y_out", [B, TWIN if TT == T else TT, V], F32,
                           kind="ExternalOutput")
    l_out = nc.dram_tensor("l_out", [B, TWIN if TT == T else TT, V], F32,
                           kind="ExternalOutput")

    ydram = nc.dram_tensor("ydram", [B, TT, V], F32)
    ldram = nc.dram_tensor("ldram", [B, TT, V], F32)
    din1 = [nc.dram_tensor(f"din1{p}", [128, B], FP16) for p in range(2)]
    dout1 = [nc.dram_tensor(f"dout1{p}", [U, B], FP16, addr_space="Shared")
             for p in range(2)]
    din23 = [nc.dram_tensor(f"din23{p}", [128, 2, B], FP16) for p in range(2)]
    dout23 = [nc.dram_tensor(f"dout23{p}", [U, 2, B], FP16,
                             addr_space="Shared") for p in range(2)]
    RG = [list(range(NCORE))]

    XCH = 32  # x chunk steps
    ORD = {0: 0, 1: 2, 2: 1}  # region order within a slot: [h1 | h3 | h2]

    with TileContext(nc) as tc:
        with (
            tc.tile_pool(name="wpool", bufs=1) as wpool,
            tc.tile_pool(name="state", bufs=1) as state,
            tc.tile_pool(name="work", bufs=2) as work,
            tc.tile_pool(name="ps", bufs=2, space="PSUM") as ps,
            tc.tile_pool(name="pslb", bufs=1, space="PSUM") as pslb,
            tc.tile_pool(name="pstr", bufs=2, space="PSUM") as pstr,
        ):
            # ---- resident weights
            def load_w(name, dram, kt, ncol):
                t = wpool.tile([128, kt, ncol], FP16, name=name, tag=name)
                nc.sync.dma_start(
                    t[:, :, :], dram.ap().rearrange("(k p) g -> p k g", p=128))
                return t

            rk1s = load_w("rk1s", rk1, KT, GS)
            rk2s = load_w("rk2s", rk2, KT, GS)
            rk3s = load_w("rk3s", rk3, KT, GS)
            k2s = load_w("k2s", k2, KT, GS)
            k3s = load_w("k3s", k3, KT, GS)
            ws = load_w("ws", wp, KT, V)
            k1Ls = load_w("k1Ls", k1L, L // 128, GS)
            latTs = load_w("latTs", latT, L // 128, B)
            k1xs = wpool.tile([F, GS], FP16, tag="k1xs")
            nc.sync.dma_start(k1xs[:, :], k1x[:, :])
            e64 = wpool.tile([B, B], FP16, tag="e64")
            nc.sync.dma_start(e64[:, :], eye64[:, :])
            e128 = wpool.tile([128, 128], FP16, tag="e128")
            nc.sync.dma_start(e128[:, :], eye128[:, :])
            on1 = wpool.tile([1, B], FP16, tag="on1")
            nc.sync.dma_start(on1[:, :], ones1[:, :])
            b2s = wpool.tile([1, GS], FP16, tag="b2s")
            nc.sync.dma_start(b2s[:, :], b2r[:, :])
            b3s = wpool.tile([1, GS], FP16, tag="b3s")
            nc.sync.dma_start(b3s[:, :], b3r[:, :])
            bvs = wpool.tile([1, V], FP16, tag="bvs")
            nc.sync.dma_start(bvs[:, :], bvr[:, :])
            b1s = wpool.tile([1, GS], FP16, tag="b1s")
            nc.sync.dma_start(b1s[:, :], b1r[:, :])

            # ---- latentb = latT.T @ k1L + b1  (once)
            lb_ps = pslb.tile([B, GS], F32, tag="lbps")
            for k in range(L // 128):
                nc.tensor.matmul(lb_ps[:, :], latTs[:, k, :], k1Ls[:, k, :],
                                 start=(k == 0), stop=False)
            nc.tensor.matmul(lb_ps[:, :], on1[:, :], b1s[:, :],
                             start=False, stop=True)
            latentb = state.tile([B, GS], FP16, tag="latentb")
            nc.vector.tensor_copy(latentb[:, :], lb_ps[:, :])

            # ---- states
            cAB = state.tile([128, US], F32, name="cAB", tag="cAB")
            nc.vector.memset(cAB[:, :], 0.0)
            c3t = state.tile([B, US], F32, name="c3t", tag="c3t")
            nc.vector.memset(c3t[:, :], 0.0)
            # h1a: gathered h1 [128, k, B]; h23a: gathered {h3, h2}
            # [128, l, k, B] (l=0 h3, l=1 h2). Separate tiles so phase-a
            # matmuls (h1 consumers) never wait on the AG23 DMA.
            h1a = [state.tile([128, KT * B], FP16, name=f"h1a{p}",
                              tag=f"h1a{p}") for p in range(2)]
            h23a = [state.tile([128, 2 * KT * B], FP16, name=f"h23a{p}",
                               tag=f"h23a{p}") for p in range(2)]
            for h in h1a + h23a:
                nc.vector.memset(h[:, :].bitcast(F32), 0.0)
            evac1 = [state.tile([128, B], FP16, name=f"evac1{p}",
                                tag=f"evac1{p}") for p in range(2)]
            evac23 = [state.tile([128, 2 * B], FP16, name=f"evac23{p}",
                                 tag=f"evac23{p}") for p in range(2)]
            for e in evac1 + evac23:
                nc.vector.memset(e[:, :].bitcast(F32), 0.0)

            def h1_slice(hbuf, k):
                return hbuf[:, k * B:(k + 1) * B]

            def h23_slice(hbuf, layer, k):
                # layer 1 -> l=1 (h2), layer 2 -> l=0 (h3)
                l = 0 if layer == 2 else 1
                off = (l * KT + k) * B
                return hbuf[:, off:off + B]

            xsb = [state.tile([F, XCH, B], FP16, name=f"xsb{p}", tag=f"xsb{p}")
                   for p in range(2)]

            nsteps = TT + 3
            for s in range(nsteps):
                par = s % 2
                hp1 = h1a[(s + 1) % 2]    # written by AG1(s-1)
                hp23 = h23a[(s + 1) % 2]  # written by AG23(s-1)
                t1, t2, t3, tpj = s, s - 1, s - 2, s - 3
                do1 = 0 <= t1 < TT
                do2 = 0 <= t2 < TT
                do3 = 0 <= t3 < TT
                dopj = 0 <= tpj < TT

                psAB = ps.tile([128, GS], F32, name=f"psAB{s}", tag="psAB")
                psB = ps.tile([128, V], F32, name=f"psB{s}", tag="psB")

                if do1 and t1 % XCH == 0:
                    xb = xsb[(t1 // XCH) % 2]
                    hi = min(XCH, TT - t1)
                    nc.sync.dma_start(xb[:, 0:hi, :], xT[:, t1:t1 + hi, :])

                # ---- emission queues. *_a needs only bcast1 round cnt1 /
                # older data; *_b needs bcast23 round cnt23.
                cg0_a, cg0_b = [], []    # psum partitions 0:64
                cg64_a, cg64_b = [], []  # psum partitions 64:128
                if do1:
                    g1 = psAB[0:B, :]
                    for k in range(KT):
                        cg0_a.append(lambda k=k: nc.tensor.matmul(
                            g1, h1_slice(hp1, k), rk1s[:, k, :],
                            start=(k == 0), stop=False))
                    xb = xsb[(t1 // XCH) % 2]
                    cg0_a.append(lambda: nc.tensor.matmul(
                        g1, xb[:, t1 % XCH, :], k1xs[:, :],
                        start=False, stop=False))
                    cg0_a.append(lambda: nc.tensor.matmul(
                        g1, e64[:, :], latentb[:, :], start=False, stop=True))
                if do2:
                    g2 = psAB[B:128, :]
                    for k in range(KT):
                        # h1(s-1): bcast1 round cnt1
                        cg64_a.append(lambda k=k: nc.tensor.matmul(
                            g2, h1_slice(hp1, k), k2s[:, k, :],
                            start=(k == 0), stop=False))
                    for k in range(KT):
                        # h2(s-2): bcast23 round cnt23
                        cg64_b.append(lambda k=k: nc.tensor.matmul(
                            g2, h23_slice(hp23, 1, k), rk2s[:, k, :],
                            start=False, stop=False))
                    cg64_b.append(lambda: nc.tensor.matmul(
                        g2, on1[:, :], b2s[:, :], start=False, stop=True))
                if dopj:
                    gp = psB[B:128, :]
                    cg64_b.append(lambda: nc.tensor.matmul(
                        gp, on1[:, :], bvs[:, :], start=True, stop=False))
                    for k in range(KT):
                        # h3(s-3): bcast23 round cnt23
                        cg64_b.append(lambda k=k: nc.tensor.matmul(
                            gp, h23_slice(hp23, 2, k), ws[:, k, :],
                            start=False, stop=(k == KT - 1)))
                if do3:
                    g3 = psB[0:B, :]
                    for k in range(KT):
                        # h3(s-3): bcast23 round cnt23
                        cg0_b.append(lambda k=k: nc.tensor.matmul(
                            g3, h23_slice(hp23, 2, k),
                            rk3s[:, k, :], start=(k == 0), stop=False))
                    cg0_b.append(lambda: nc.tensor.matmul(
                        g3, on1[:, :], b3s[:, :], start=False, stop=False))
                    for k in range(KT):
                        # h2(s-2): round cnt23
                        cg0_b.append(lambda k=k: nc.tensor.matmul(
                            g3, h23_slice(hp23, 1, k), k3s[:, k, :],
                            start=False, stop=(k == KT - 1)))

                gsbA = work.tile([128, GS], F32, name=f"gsbA{s}", tag="gsbA")
                gsb3 = work.tile([B, GS], F32, name=f"gsb3{s}", tag="gsb3")
                hsb1 = work.tile([B, US], FP16, name=f"h1_{s}", tag="h1")
                hsb23 = work.tile([128, US], FP16, name=f"h23_{s}", tag="h23")
                tpC = pstr.tile([128, 192], FP16, name=f"tpC_{s}", tag="tpC")
                tp1 = tpC[:, 0:64]
                tp23 = tpC[:, 64:192]

                tmpAB = work.tile([128, US], F32, name=f"tmpAB{s}",
                                  tag="tmpAB")
                thAB = work.tile([128, US], F32, name=f"thAB{s}", tag="thAB")
                tmp3 = work.tile([B, US], F32, name=f"tmp3_{s}", tag="tmp3")
                th3 = work.tile([B, US], F32, name=f"th3_{s}", tag="th3")

                def cell_vec(gs, c, hout, tmp, th):
                    nc.vector.tensor_tensor(tmp, gs[:, 0:128],
                                            gs[:, 384:512], ALU.mult)
                    nc.vector.tensor_tensor(c, gs[:, 128:256], c, ALU.mult)
                    nc.vector.tensor_tensor(c, c, tmp, ALU.add)
                    nc.scalar.activation(th, c, ACT.Tanh)
                    nc.vector.tensor_tensor(hout, gs[:, 256:384], th,
                                            ALU.mult)

                def l1_act():
                    g1 = psAB[0:B, :]
                    nc.scalar.activation(gsbA[0:B, 0:384], g1[:, 0:384],
                                         ACT.Sigmoid)
                    nc.scalar.activation(gsbA[0:B, 384:512], g1[:, 384:512],
                                         ACT.Tanh)
                    cell_vec(gsbA[0:B, :], cAB[0:B, :], hsb1[:, :],
                             tmpAB[0:B, :], thAB[0:B, :])

                # ---- phase a (only needs AG1(s-1) / older)
                na = len(cg0_a)  # 10 when do1
                for i in range(na):
                    cg0_a[i]()
                    if do1 and i == na - 1:
                        l1_act()
                    if i < len(cg64_a):
                        cg64_a[i]()

                # ---- phase b (needs AG23(s-1)); tp1 + AG1 emission slots
                # in a few pairs deep so the act chain has time to finish.
                rest64 = cg64_a[na:] + cg64_b
                nb = max(len(cg0_b), len(rest64))
                ag1_emitted = False

                def emit_ag1():
                    nc.tensor.transpose(tp1, hsb1[:, :], e64[:, :])
                    nc.vector.tensor_copy(evac1[par][:, :], tp1)
                    nc.sync.dma_start(din1[par].ap(), evac1[par][:, :])
                    nc.gpsimd.collective_compute(
                        "AllGather", ALU.bypass, replica_groups=RG,
                        ins=[din1[par].ap().opt()],
                        outs=[dout1[par].ap().opt()],
                    )
                    nc.sync.dma_start(
                        h1a[par][:, :].rearrange("p (k b) -> p k b", k=KT),
                        dout1[par].ap().rearrange("(k p) b -> p k b", p=128),
                    )

                for i in range(nb):
                    if i < len(cg0_b):
                        cg0_b[i]()
                    if do1 and i == 3:
                        emit_ag1()
                        ag1_emitted = True
                    if i < len(rest64):
                        rest64[i]()
                if do1 and not ag1_emitted:
                    emit_ag1()

                # ---- L2 / L3 activations
                if do2:
                    g2 = psAB[B:128, :]
                    nc.scalar.activation(gsbA[B:128, 0:384], g2[:, 0:384],
                                         ACT.Sigmoid)
                    nc.scalar.activation(gsbA[B:128, 384:512], g2[:, 384:512],
                                         ACT.Tanh)
                    cell_vec(gsbA[B:128, :], cAB[B:128, :], hsb23[B:128, :],
                             tmpAB[B:128, :], thAB[B:128, :])
                if do3:
                    g3 = psB[0:B, :]
                    nc.scalar.activation(gsb3[:, 0:384], g3[:, 0:384],
                                         ACT.Sigmoid)
                    nc.scalar.activation(gsb3[:, 384:512], g3[:, 384:512],
                                         ACT.Tanh)
                    cell_vec(gsb3[:, :], c3t[:, :], hsb23[0:B, :],
                             tmp3[:, :], th3[:, :])

                # ---- broadcast {h3, h2}
                if do2 or do3:
                    if not do3:
                        nc.vector.memset(hsb23[0:B, :].bitcast(F32), 0.0)
                    if not do2:
                        nc.vector.memset(hsb23[B:128, :].bitcast(F32), 0.0)
                    nc.tensor.transpose(tp23, hsb23[:, :], e128[:, :])
                    nc.vector.tensor_copy(evac23[par][:, :], tp23)
                    nc.sync.dma_start(
                        din23[par].ap().rearrange("p l b -> p (l b)"),
                        evac23[par][:, :])
                    nc.gpsimd.collective_compute(
                        "AllGather", ALU.bypass, replica_groups=RG,
                        ins=[din23[par].ap().opt()],
                        outs=[dout23[par].ap().opt()],
                    )
                    nc.sync.dma_start(
                        h23a[par][:, :].rearrange("p (l k b) -> p l k b",
                                                  l=2, k=KT),
                        dout23[par].ap().rearrange("(k p) l b -> p l k b",
                                                   p=128),
                    )

                # ---- softmax + outputs for step tpj
                if dopj:
                    gp = psB[B:128, :]
                    lsb = work.tile([128, V], F32, name=f"lsb{tpj}", tag="lsb")
                    nc.vector.tensor_copy(lsb[B:128, :], gp)
                    nmx = work.tile([128, 1], F32, name=f"nmx{tpj}", tag="nmx")
                    nc.vector.tensor_reduce(nmx[B:128, :], lsb[B:128, :],
                                            axis=mybir.AxisListType.X,
                                            op=ALU.max, negate=True)
                    esb = work.tile([128, V], F32, name=f"esb{tpj}", tag="esb")
                    nc.scalar.activation(esb[B:128, :], lsb[B:128, :], ACT.Exp,
                                         bias=nmx[B:128, 0:1])
                    sm = work.tile([128, 1], F32, name=f"sm{tpj}", tag="sm")
                    nc.vector.tensor_reduce(sm[B:128, :], esb[B:128, :],
                                            axis=mybir.AxisListType.X,
                                            op=ALU.add)
                    rs = work.tile([128, 1], F32, name=f"rs{tpj}", tag="rs")
                    nc.vector.reciprocal(rs[B:128, :], sm[B:128, :])
                    ysb = work.tile([128, V], F32, name=f"ysb{tpj}", tag="ysb")
                    nc.vector.tensor_scalar(ysb[B:128, :], esb[B:128, :],
                                            rs[B:128, 0:1], None, ALU.mult)
                    nc.sync.dma_start(ydram[:, tpj:tpj + 1, :], ysb[B:128, :])
                    nc.sync.dma_start(ldram[:, tpj:tpj + 1, :], lsb[B:128, :])

            # windowed output copy (per-core time window)
            if TT == T:
                pid = nc.gpsimd.partition_id()
                off = pid * TWIN
                nc.gpsimd.dma_start(y_out[:, :, :],
                                    ydram[:, bass.ds(off, TWIN), :])
                nc.gpsimd.dma_start(l_out[:, :, :],
                                    ldram[:, bass.ds(off, TWIN), :])
            else:
                nc.gpsimd.dma_start(y_out[:, :, :], ydram[:, :, :])
                nc.gpsimd.dma_start(l_out[:, :, :], ldram[:, :, :])

    nc.compile()
    return nc


_built = {}


def _get_nc(t_steps):
    if t_steps not in _built:
        _built[t_steps] = build(t_steps)
    return _built[t_steps]


def _prep_inputs(latent, x, k1, rk1, b1, k2, rk2, b2, k3, rk3, b3, w, b,
                 t_steps):
    f16 = np.float16
    latent = np.asarray(latent, f16)
    x = np.asarray(x, f16)
    k1 = np.asarray(k1, f16)
    rk1 = np.asarray(rk1, f16)
    k2 = np.asarray(k2, f16)
    rk2 = np.asarray(rk2, f16)
    k3 = np.asarray(k3, f16)
    rk3 = np.asarray(rk3, f16)
    w = np.asarray(w, f16)
    in_maps = []
    latT = np.ascontiguousarray(latent.T)                   # [L, B]
    xT = np.ascontiguousarray(np.transpose(x, (2, 1, 0)))   # [F, T, B]
    eye64v = np.eye(B, dtype=f16)
    eye128v = np.eye(128, dtype=f16)
    ones1v = np.ones((1, B), f16)
    for j in range(NCORE):
        u0 = j * US
        cols = np.concatenate([
            np.arange(u0, u0 + US),                  # i
            np.arange(U + u0, U + u0 + US),          # f
            np.arange(3 * U + u0, 3 * U + u0 + US),  # o
            np.arange(2 * U + u0, 2 * U + u0 + US),  # g
        ])
        in_maps.append({
            "latT": latT,
            "xT": np.ascontiguousarray(xT[:, :t_steps, :]),
            "k1L": np.ascontiguousarray(k1[:L, cols]),
            "k1x": np.ascontiguousarray(k1[L:, cols]),
            "rk1": np.ascontiguousarray(rk1[:, cols]),
            "k2": np.ascontiguousarray(k2[:, cols]),
            "rk2": np.ascontiguousarray(rk2[:, cols]),
            "k3": np.ascontiguousarray(k3[:, cols]),
            "rk3": np.ascontiguousarray(rk3[:, cols]),
            "wp": np.ascontiguousarray(np.asarray(w, f16)),
            "b1r": np.ascontiguousarray(np.asarray(b1, f16)[cols][None, :]),
            "b2r": np.ascontiguousarray(np.asarray(b2, f16)[cols][None, :]),
            "b3r": np.ascontiguousarray(np.asarray(b3, f16)[cols][None, :]),
            "bvr": np.ascontiguousarray(np.asarray(b, f16)[None, :]),
            "eye64": eye64v,
            "eye128": eye128v,
            "ones1": ones1v,
        })
    return in_maps


def run(t_steps=T, **inputs):
    nc = _get_nc(t_steps)
    in_maps = _prep_inputs(t_steps=t_steps, **inputs)
    res = run_bass_kernel_spmd(nc, in_maps, core_ids=list(range(NCORE)))
    if t_steps == T:
        y = np.concatenate([res.results[j]["y_out"] for j in range(NCORE)],
                           axis=1)
        lg = np.concatenate([res.results[j]["l_out"] for j in range(NCORE)],
                            axis=1)
    else:
        y = res.results[0]["y_out"]
        lg = res.results[0]["l_out"]
    return y, lg


def kernel(**inputs):
    return run(t_steps=T, **inputs)


# revision 3
# speedup vs baseline: 1.0229x; 1.0229x over previous
"""3-layer LSTM decoder + projection + softmax on 8 trn2 NeuronCores. v3.

Like v2 (PE column-group paired matmuls), but fp16 operands (bf16
drifts past the 2e-2 gate over 256 steps) and the per-step h exchange
uses remote_dma_broadcast (direct SBUF->SBUF push to all 8 cores via
the 16 SDMA engines) instead of ncfw AllGather collectives (~8-14us
each, serialized on the collective cores).

Each core broadcasts its transposed h-slices into every core's hrx
buffer at slot <pid>; slots double as matmul contract k-tiles, so data
lands matmul-ready. Arrival is signaled by remote semaphores (each
sender's broadcast adds 2 per receiver; 16 per full round). Receivers
wait_ge on the PE queue before consuming. hrx is triple-buffered: a
sender can only reach step s+3 after all cores passed their step s+2
waits, which (PE in-order) implies step s+1 reads are done -> no WAR
race.

Step s emission: [wait rsem1>=16*cnt1] phase-a matmuls (rk1/x/lat |
proj/k2h1 head) which need only h1(s-1) or older; h1 evac + broadcast;
[wait rsem23>=16*cnt23] phase-b matmuls (rk3/b3/k3h2 | k2h1 tail/
rk2/b2) which need h2(s-2)/h3(s-3... fresh round); h2/h3 evac +
broadcast; softmax/outputs.
"""
import numpy as np

import concourse.bass as bass
import concourse.bacc as bacc
import concourse.mybir as mybir
from concourse.tile import TileContext
from concourse.bass_utils import run_bass_kernel_spmd

F32 = mybir.dt.float32
FP16 = mybir.dt.float16
ACT = mybir.ActivationFunctionType
ALU = mybir.AluOpType

B, T, U, V, L, F = 64, 256, 1024, 512, 512, 64
NCORE = 8
GS = 512          # gates per core (4 x 128 units)
US = 128          # units per core
KT = U // 128     # 8 contract tiles over hidden
TWIN = T // NCORE
SLOT = 3 * B      # per-sender slot in hrx: [h1 | h3 | h2] x B columns


def build(t_steps=T):
    nc = bacc.Bacc("TRN2", target_bir_lowering=False)
    TT = t_steps

    latT = nc.dram_tensor("latT", [L, B], FP16, kind="ExternalInput")
    xT = nc.dram_tensor("xT", [F, TT, B], FP16, kind="ExternalInput")
    k1L = nc.dram_tensor("k1L", [L, GS], FP16, kind="ExternalInput")
    k1x = nc.dram_tensor("k1x", [F, GS], FP16, kind="ExternalInput")
    rk1 = nc.dram_tensor("rk1", [U, GS], FP16, kind="ExternalInput")
    k2 = nc.dram_tensor("k2", [U, GS], FP16, kind="ExternalInput")
    rk2 = nc.dram_tensor("rk2", [U, GS], FP16, kind="ExternalInput")
    k3 = nc.dram_tensor("k3", [U, GS], FP16, kind="ExternalInput")
    rk3 = nc.dram_tensor("rk3", [U, GS], FP16, kind="ExternalInput")
    wp = nc.dram_tensor("wp", [U, V], FP16, kind="ExternalInput")
    b1r = nc.dram_tensor("b1r", [1, GS], FP16, kind="ExternalInput")
    b2r = nc.dram_tensor("b2r", [1, GS], FP16, kind="ExternalInput")
    b3r = nc.dram_tensor("b3r", [1, GS], FP16, kind="ExternalInput")
    bvr = nc.dram_tensor("bvr", [1, V], FP16, kind="ExternalInput")
    eye64 = nc.dram_tensor("eye64", [B, B], FP16, kind="ExternalInput")
    eye128 = nc.dram_tensor("eye128", [128, 128], FP16, kind="ExternalInput")
    ones1 = nc.dram_tensor("ones1", [1, B], FP16, kind="ExternalInput")

    y_out = nc.dram_tensor("y_out", [B, TWIN if TT == T else TT, V], F32,
                           kind="ExternalOutput")
    l_out = nc.dram_tensor("l_out", [B, TWIN if TT == T else TT, V], F32,
                           kind="ExternalOutput")

    ydram = nc.dram_tensor("ydram", [B, TT, V], F32)
    ldram = nc.dram_tensor("ldram", [B, TT, V], F32)
    din1 = [nc.dram_tensor(f"din1{p}", [128, B], FP16) for p in range(2)]
    dout1 = [nc.dram_tensor(f"dout1{p}", [U, B], FP16, addr_space="Shared")
             for p in range(2)]
    din23 = [nc.dram_tensor(f"din23{p}", [128, 2, B], FP16) for p in range(2)]
    dout23 = [nc.dram_tensor(f"dout23{p}", [U, 2, B], FP16,
                             addr_space="Shared") for p in range(2)]
    RG = [list(range(NCORE))]

    XCH = 32  # x chunk steps
    ORD = {0: 0, 1: 2, 2: 1}  # region order within a slot: [h1 | h3 | h2]

    with TileContext(nc) as tc:
        with (
            tc.tile_pool(name="wpool", bufs=1) as wpool,
            tc.tile_pool(name="state", bufs=1) as state,
            tc.tile_pool(name="work", bufs=2) as work,
            tc.tile_pool(name="ps", bufs=2, space="PSUM") as ps,
            tc.tile_pool(name="pslb", bufs=1, space="PSUM") as pslb,
            tc.tile_pool(name="pstr", bufs=2, space="PSUM") as pstr,
        ):
            # ---- resident weights
            def load_w(name, dram, kt, ncol):
                t = wpool.tile([128, kt, ncol], FP16, name=name, tag=name)
                nc.sync.dma_start(
                    t[:, :, :], dram.ap().rearrange("(k p) g -> p k g", p=128))
                return t

            rk1s = load_w("rk1s", rk1, KT, GS)
            rk2s = load_w("rk2s", rk2, KT, GS)
            rk3s = load_w("rk3s", rk3, KT, GS)
            k2s = load_w("k2s", k2, KT, GS)
            k3s = load_w("k3s", k3, KT, GS)
            ws = load_w("ws", wp, KT, V)
            k1Ls = load_w("k1Ls", k1L, L // 128, GS)
            latTs = load_w("latTs", latT, L // 128, B)
            k1xs = wpool.tile([F, GS], FP16, tag="k1xs")
            nc.sync.dma_start(k1xs[:, :], k1x[:, :])
            e64 = wpool.tile([B, B], FP16, tag="e64")
            nc.sync.dma_start(e64[:, :], eye64[:, :])
            e128 = wpool.tile([128, 128], FP16, tag="e128")
            nc.sync.dma_start(e128[:, :], eye128[:, :])
            on1 = wpool.tile([1, B], FP16, tag="on1")
            nc.sync.dma_start(on1[:, :], ones1[:, :])
            b2s = wpool.tile([1, GS], FP16, tag="b2s")
            nc.sync.dma_start(b2s[:, :], b2r[:, :])
            b3s = wpool.tile([1, GS], FP16, tag="b3s")
            nc.sync.dma_start(b3s[:, :], b3r[:, :])
            bvs = wpool.tile([1, V], FP16, tag="bvs")
            nc.sync.dma_start(bvs[:, :], bvr[:, :])
            b1s = wpool.tile([1, GS], FP16, tag="b1s")
            nc.sync.dma_start(b1s[:, :], b1r[:, :])

            # ---- latentb = latT.T @ k1L + b1  (once)
            lb_ps = pslb.tile([B, GS], F32, tag="lbps")
            for k in range(L // 128):
                nc.tensor.matmul(lb_ps[:, :], latTs[:, k, :], k1Ls[:, k, :],
                                 start=(k == 0), stop=False)
            nc.tensor.matmul(lb_ps[:, :], on1[:, :], b1s[:, :],
                             start=False, stop=True)
            latentb = state.tile([B, GS], FP16, tag="latentb")
            nc.vector.tensor_copy(latentb[:, :], lb_ps[:, :])
            psD = pslb.tile([128, GS], F32, name="psD", tag="psD")

            # ---- states
            cAB = state.tile([128, US], F32, name="cAB", tag="cAB")
            nc.vector.memset(cAB[:, :], 0.0)
            c3t = state.tile([B, US], F32, name="c3t", tag="c3t")
            nc.vector.memset(c3t[:, :], 0.0)
            # h1a: gathered h1 [128, k, B]; h23a: gathered {h3, h2}
            # [128, l, k, B] (l=0 h3, l=1 h2). Separate tiles so phase-a
            # matmuls (h1 consumers) never wait on the AG23 DMA.
            h1a = [state.tile([128, KT * B], FP16, name=f"h1a{p}",
                              tag=f"h1a{p}") for p in range(2)]
            h23a = [state.tile([128, 2 * KT * B], FP16, name=f"h23a{p}",
                               tag=f"h23a{p}") for p in range(2)]
            for h in h1a + h23a:
                nc.vector.memset(h[:, :].bitcast(F32), 0.0)
            evac1 = [state.tile([128, B], FP16, name=f"evac1{p}",
                                tag=f"evac1{p}") for p in range(2)]
            evac23 = [state.tile([128, 2 * B], FP16, name=f"evac23{p}",
                                 tag=f"evac23{p}") for p in range(2)]
            for e in evac1 + evac23:
                nc.vector.memset(e[:, :].bitcast(F32), 0.0)

            def h1_slice(hbuf, k):
                return hbuf[:, k * B:(k + 1) * B]

            def h23_slice(hbuf, layer, k):
                # layer 1 -> l=1 (h2), layer 2 -> l=0 (h3)
                l = 0 if layer == 2 else 1
                off = (l * KT + k) * B
                return hbuf[:, off:off + B]

            xsb = [state.tile([F, XCH, B], FP16, name=f"xsb{p}", tag=f"xsb{p}")
                   for p in range(2)]

            nsteps = TT + 4
            for s in range(nsteps):
                par = s % 2
                hp1 = h1a[(s + 1) % 2]    # written by AG1(s-1)
                hp23 = h23a[(s + 1) % 2]  # written by AG23(s-1)
                hp23o = h23a[s % 2]       # written by AG23(s-2) (old)
                t1, t2, t3, tpj = s, s - 1, s - 2, s - 4
                do1 = 0 <= t1 < TT
                do2 = 0 <= t2 < TT
                do3 = 0 <= t3 < TT
                dopj = 0 <= tpj < TT

                psAB = ps.tile([128, GS], F32, name=f"psAB{s}", tag="psAB")
                psB = ps.tile([128, V], F32, name=f"psB{s}", tag="psB")

                if do1 and t1 % XCH == 0:
                    xb = xsb[(t1 // XCH) % 2]
                    hi = min(XCH, TT - t1)
                    nc.sync.dma_start(xb[:, 0:hi, :], xT[:, t1:t1 + hi, :])

                # ---- emission queues. *_a needs only bcast1 round cnt1 /
                # older data; *_b needs bcast23 round cnt23.
                cg0_a, cg0_b = [], []    # psum partitions 0:64
                cg64_a, cg64_b = [], []  # psum partitions 64:128
                if do1:
                    g1 = psAB[0:B, :]
                    for k in range(KT):
                        cg0_a.append(lambda k=k: nc.tensor.matmul(
                            g1, h1_slice(hp1, k), rk1s[:, k, :],
                            start=(k == 0), stop=False))
                    xb = xsb[(t1 // XCH) % 2]
                    cg0_a.append(lambda: nc.tensor.matmul(
                        g1, xb[:, t1 % XCH, :], k1xs[:, :],
                        start=False, stop=False))
                    cg0_a.append(lambda: nc.tensor.matmul(
                        g1, e64[:, :], latentb[:, :], start=False, stop=True))
                if dopj:
                    gp = psB[B:128, :]
                    cg64_a.append(lambda: nc.tensor.matmul(
                        gp, on1[:, :], bvs[:, :], start=True, stop=False))
                    for k in range(KT):
                        # h3(s-4): AG23(s-2) -> no fresh dependency
                        cg64_a.append(lambda k=k: nc.tensor.matmul(
                            gp, h23_slice(hp23o, 2, k), ws[:, k, :],
                            start=False, stop=(k == KT - 1)))
                if do2:
                    g2 = psAB[B:128, :]
                    for k in range(KT):
                        # h1(s-1): AG1(s-1)
                        cg64_a.append(lambda k=k: nc.tensor.matmul(
                            g2, h1_slice(hp1, k), k2s[:, k, :],
                            start=(k == 0), stop=False))
                    for k in range(KT):
                        # h2(s-2): bcast23 round cnt23
                        cg64_b.append(lambda k=k: nc.tensor.matmul(
                            g2, h23_slice(hp23, 1, k), rk2s[:, k, :],
                            start=False, stop=False))
                    cg64_b.append(lambda: nc.tensor.matmul(
                        g2, on1[:, :], b2s[:, :], start=False, stop=True))
                if do3:
                    g3 = psB[0:B, :]
                    for k in range(KT):
                        # h3(s-3): bcast23 round cnt23
                        cg0_b.append(lambda k=k: nc.tensor.matmul(
                            g3, h23_slice(hp23, 2, k),
                            rk3s[:, k, :], start=(k == 0), stop=False))
                    cg0_b.append(lambda: nc.tensor.matmul(
                        g3, on1[:, :], b3s[:, :], start=False, stop=False))
                    for k in range(KT):
                        # h2(s-2): round cnt23
                        cg0_b.append(lambda k=k: nc.tensor.matmul(
                            g3, h23_slice(hp23, 1, k), k3s[:, k, :],
                            start=False, stop=(k == KT - 1)))

                gsbA = work.tile([128, GS], F32, name=f"gsbA{s}", tag="gsbA")
                gsb3 = work.tile([B, GS], F32, name=f"gsb3{s}", tag="gsb3")
                hsb1 = work.tile([B, US], FP16, name=f"h1_{s}", tag="h1")
                hsb23 = work.tile([128, US], FP16, name=f"h23_{s}", tag="h23")
                tpC = pstr.tile([128, 192], FP16, name=f"tpC_{s}", tag="tpC")
                tp1 = tpC[:, 0:64]
                tp23 = tpC[:, 64:192]

                tmpAB = work.tile([128, US], F32, name=f"tmpAB{s}",
                                  tag="tmpAB")
                thAB = work.tile([128, US], F32, name=f"thAB{s}", tag="thAB")
                tmp3 = work.tile([B, US], F32, name=f"tmp3_{s}", tag="tmp3")
                th3 = work.tile([B, US], F32, name=f"th3_{s}", tag="th3")

                def cell_vec(gs, c, hout, tmp, th):
                    nc.vector.tensor_tensor(tmp, gs[:, 0:128],
                                            gs[:, 384:512], ALU.mult)
                    nc.vector.tensor_tensor(c, gs[:, 128:256], c, ALU.mult)
                    nc.vector.tensor_tensor(c, c, tmp, ALU.add)
                    nc.scalar.activation(th, c, ACT.Tanh)
                    nc.vector.tensor_tensor(hout, gs[:, 256:384], th,
                                            ALU.mult)

                def l1_act():
                    g1 = psAB[0:B, :]
                    nc.scalar.activation(gsbA[0:B, 0:384], g1[:, 0:384],
                                         ACT.Sigmoid)
                    nc.scalar.activation(gsbA[0:B, 384:512], g1[:, 384:512],
                                         ACT.Tanh)
                    cell_vec(gsbA[0:B, :], cAB[0:B, :], hsb1[:, :],
                             tmpAB[0:B, :], thAB[0:B, :])

                # ---- phase a (needs only AG1(s-1) / AG23(s-2)); runs
                # inside AG23(s-1)'s span.
                na = len(cg0_a)  # 10 when do1
                nA = max(na, len(cg64_a))
                for i in range(nA):
                    if i < na:
                        cg0_a[i]()
                    if do1 and i == na - 1:
                        l1_act()
                    if i < len(cg64_a):
                        cg64_a[i]()

                # AG1 chain emitted immediately: its trigger is data-ready
                # mid-AG23(s-1), so the collective core takes it the moment
                # it frees.
                if do1:
                    nc.tensor.transpose(tp1, hsb1[:, :], e64[:, :])
                    nc.vector.tensor_copy(evac1[par][:, :], tp1)
                    nc.sync.dma_start(din1[par].ap(), evac1[par][:, :])
                    nc.gpsimd.collective_compute(
                        "AllGather", ALU.bypass, replica_groups=RG,
                        ins=[din1[par].ap().opt()],
                        outs=[dout1[par].ap().opt()],
                    )
                    nc.sync.dma_start(
                        h1a[par][:, :].rearrange("p (k b) -> p k b", k=KT),
                        dout1[par].ap().rearrange("(k p) b -> p k b", p=128),
                    )

                # PE warm-keepers: bridge the idle gap until AG23(s-1)
                # lands so HAM stays at K=8/8 (idle > ~3.4us rethrottles).
                if 0 < s < TT:
                    for _ in range(5):
                        nc.tensor.matmul(psD[0:B, :], e64[:, :],
                                         latentb[:, :], start=True, stop=True)
                        nc.tensor.matmul(psD[B:128, :], e64[:, :],
                                         latentb[:, :], start=True, stop=True)

                # ---- phase b (needs AG23(s-1))
                rest64 = cg64_b
                nb = max(len(cg0_b), len(rest64))
                for i in range(nb):
                    if i < len(cg0_b):
                        cg0_b[i]()
                    if i < len(rest64):
                        rest64[i]()

                # ---- L2 / L3 activations
                if do2:
                    g2 = psAB[B:128, :]
                    nc.scalar.activation(gsbA[B:128, 0:384], g2[:, 0:384],
                                         ACT.Sigmoid)
                    nc.scalar.activation(gsbA[B:128, 384:512], g2[:, 384:512],
                                         ACT.Tanh)
                    cell_vec(gsbA[B:128, :], cAB[B:128, :], hsb23[B:128, :],
                             tmpAB[B:128, :], thAB[B:128, :])
                if do3:
                    g3 = psB[0:B, :]
                    nc.scalar.activation(gsb3[:, 0:384], g3[:, 0:384],
                                         ACT.Sigmoid)
                    nc.scalar.activation(gsb3[:, 384:512], g3[:, 384:512],
                                         ACT.Tanh)
                    cell_vec(gsb3[:, :], c3t[:, :], hsb23[0:B, :],
                             tmp3[:, :], th3[:, :])

                # ---- broadcast {h3, h2}
                if do2 or do3:
                    if not do3:
                        nc.vector.memset(hsb23[0:B, :].bitcast(F32), 0.0)
                    if not do2:
                        nc.vector.memset(hsb23[B:128, :].bitcast(F32), 0.0)
                    nc.tensor.transpose(tp23, hsb23[:, :], e128[:, :])
                    nc.vector.tensor_copy(evac23[par][:, :], tp23)
                    nc.sync.dma_start(
                        din23[par].ap().rearrange("p l b -> p (l b)"),
                        evac23[par][:, :])
                    nc.gpsimd.collective_compute(
                        "AllGather", ALU.bypass, replica_groups=RG,
                        ins=[din23[par].ap().opt()],
                        outs=[dout23[par].ap().opt()],
                    )
                    nc.sync.dma_start(
                        h23a[par][:, :].rearrange("p (l k b) -> p l k b",
                                                  l=2, k=KT),
                        dout23[par].ap().rearrange("(k p) l b -> p l k b",
                                                   p=128),
                    )

                # ---- softmax + outputs for step tpj
                if dopj:
                    gp = psB[B:128, :]
                    lsb = work.tile([128, V], F32, name=f"lsb{tpj}", tag="lsb")
                    nc.vector.tensor_copy(lsb[B:128, :], gp)
                    nmx = work.tile([128, 1], F32, name=f"nmx{tpj}", tag="nmx")
                    nc.vector.tensor_reduce(nmx[B:128, :], lsb[B:128, :],
                                            axis=mybir.AxisListType.X,
                                            op=ALU.max, negate=True)
                    esb = work.tile([128, V], F32, name=f"esb{tpj}", tag="esb")
                    nc.scalar.activation(esb[B:128, :], lsb[B:128, :], ACT.Exp,
                                         bias=nmx[B:128, 0:1])
                    sm = work.tile([128, 1], F32, name=f"sm{tpj}", tag="sm")
                    nc.vector.tensor_reduce(sm[B:128, :], esb[B:128, :],
                                            axis=mybir.AxisListType.X,
                                            op=ALU.add)
                    rs = work.tile([128, 1], F32, name=f"rs{tpj}", tag="rs")
                    nc.vector.reciprocal(rs[B:128, :], sm[B:128, :])
                    ysb = work.tile([128, V], F32, name=f"ysb{tpj}", tag="ysb")
                    nc.vector.tensor_scalar(ysb[B:128, :], esb[B:128, :],
                                            rs[B:128, 0:1], None, ALU.mult)
                    nc.sync.dma_start(ydram[:, tpj:tpj + 1, :], ysb[B:128, :])
                    nc.sync.dma_start(ldram[:, tpj:tpj + 1, :], lsb[B:128, :])

            # windowed output copy (per-core time window)
            if TT == T:
                pid = nc.gpsimd.partition_id()
                off = pid * TWIN
                nc.gpsimd.dma_start(y_out[:, :, :],
                                    ydram[:, bass.ds(off, TWIN), :])
                nc.gpsimd.dma_start(l_out[:, :, :],
                                    ldram[:, bass.ds(off, TWIN), :])
            else:
                nc.gpsimd.dma_start(y_out[:, :, :], ydram[:, :, :])
                nc.gpsimd.dma_start(l_out[:, :, :], ldram[:, :, :])

    nc.compile()
    return nc


_built = {}


def _get_nc(t_steps):
    if t_steps not in _built:
        _built[t_steps] = build(t_steps)
    return _built[t_steps]


def _prep_inputs(latent, x, k1, rk1, b1, k2, rk2, b2, k3, rk3, b3, w, b,
                 t_steps):
    f16 = np.float16
    latent = np.asarray(latent, f16)
    x = np.asarray(x, f16)
    k1 = np.asarray(k1, f16)
    rk1 = np.asarray(rk1, f16)
    k2 = np.asarray(k2, f16)
    rk2 = np.asarray(rk2, f16)
    k3 = np.asarray(k3, f16)
    rk3 = np.asarray(rk3, f16)
    w = np.asarray(w, f16)
    in_maps = []
    latT = np.ascontiguousarray(latent.T)                   # [L, B]
    xT = np.ascontiguousarray(np.transpose(x, (2, 1, 0)))   # [F, T, B]
    eye64v = np.eye(B, dtype=f16)
    eye128v = np.eye(128, dtype=f16)
    ones1v = np.ones((1, B), f16)
    for j in range(NCORE):
        u0 = j * US
        cols = np.concatenate([
            np.arange(u0, u0 + US),                  # i
            np.arange(U + u0, U + u0 + US),          # f
            np.arange(3 * U + u0, 3 * U + u0 + US),  # o
            np.arange(2 * U + u0, 2 * U + u0 + US),  # g
        ])
        in_maps.append({
            "latT": latT,
            "xT": np.ascontiguousarray(xT[:, :t_steps, :]),
            "k1L": np.ascontiguousarray(k1[:L, cols]),
            "k1x": np.ascontiguousarray(k1[L:, cols]),
            "rk1": np.ascontiguousarray(rk1[:, cols]),
            "k2": np.ascontiguousarray(k2[:, cols]),
            "rk2": np.ascontiguousarray(rk2[:, cols]),
            "k3": np.ascontiguousarray(k3[:, cols]),
            "rk3": np.ascontiguousarray(rk3[:, cols]),
            "wp": np.ascontiguousarray(np.asarray(w, f16)),
            "b1r": np.ascontiguousarray(np.asarray(b1, f16)[cols][None, :]),
            "b2r": np.ascontiguousarray(np.asarray(b2, f16)[cols][None, :]),
            "b3r": np.ascontiguousarray(np.asarray(b3, f16)[cols][None, :]),
            "bvr": np.ascontiguousarray(np.asarray(b, f16)[None, :]),
            "eye64": eye64v,
            "eye128": eye128v,
            "ones1": ones1v,
        })
    return in_maps


def run(t_steps=T, **inputs):
    nc = _get_nc(t_steps)
    in_maps = _prep_inputs(t_steps=t_steps, **inputs)
    res = run_bass_kernel_spmd(nc, in_maps, core_ids=list(range(NCORE)))
    if t_steps == T:
        y = np.concatenate([res.results[j]["y_out"] for j in range(NCORE)],
                           axis=1)
        lg = np.concatenate([res.results[j]["l_out"] for j in range(NCORE)],
                            axis=1)
    else:
        y = res.results[0]["y_out"]
        lg = res.results[0]["l_out"]
    return y, lg


def kernel(**inputs):
    return run(t_steps=T, **inputs)


# revision 5
# speedup vs baseline: 1.0275x; 1.0045x over previous
"""3-layer LSTM decoder + projection + softmax on 8 trn2 NeuronCores. v6.

Sharding: hidden units / gates sharded 8 ways (512 gates = [i|f|o|g] x
128 units per core); the time recurrence runs as a wavefront: step s
computes L1(s), L2(s-1), L3(s-2), projection(s-4). The full h vectors
are re-gathered every step with two AllGathers (AG1{h1} 16KB,
AG23{h3,h2} 32KB, fp16).

Perf structure (vs the fp32r baseline, 10.8ms -> 7.1ms):
- fp16 matmul operands (fp32r cannot use PE column tiling; bf16 drifts
  past the 2e-2 gate over 256 steps; fp16 passes at ~3e-3).
- PE column-group pairing: gate psums for L1/L3 live in psum partitions
  0:64, L2/proj in 64:128; matmuls are emitted interleaved so two M=64
  matmuls run concurrently in opposite halves of the 128x128 PE array.
- Emission order per step hides collective latency: an "early" block
  (x/latentb, projection on 2-step-old h3, warm-keeper matmuls to stop
  HAM rethrottling) runs while AG1(s-1)/AG23(s-1) are in flight; then
  rk1||k2h1 (gated on AG1(s-1)) and the h1 evac + AG1(s) chain so the
  collective core picks AG1 up as soon as AG23(s-1) ends; then phase b
  (rk3/b3/k3h2 || rk2/b2, gated on AG23(s-1)), h2/h3 evac + AG23(s),
  softmax and output DMAs.
The remaining period (~27us) is dominated by the ncfw collective spans
(~8+9us serialized on the collective cores) plus the evac->DMA->
trigger chain; compute is largely hidden under them.
"""
import numpy as np

import concourse.bass as bass
import concourse.bacc as bacc
import concourse.mybir as mybir
from concourse.tile import TileContext
from concourse.bass_utils import run_bass_kernel_spmd

F32 = mybir.dt.float32
FP16 = mybir.dt.float16
ACT = mybir.ActivationFunctionType
ALU = mybir.AluOpType

B, T, U, V, L, F = 64, 256, 1024, 512, 512, 64
NCORE = 8
GS = 512          # gates per core (4 x 128 units)
US = 128          # units per core
KT = U // 128     # 8 contract tiles over hidden
TWIN = T // NCORE
SLOT = 3 * B      # per-sender slot in hrx: [h1 | h3 | h2] x B columns


def build(t_steps=T):
    nc = bacc.Bacc("TRN2", target_bir_lowering=False)
    TT = t_steps

    latT = nc.dram_tensor("latT", [L, B], FP16, kind="ExternalInput")
    xT = nc.dram_tensor("xT", [F, TT, B], FP16, kind="ExternalInput")
    k1L = nc.dram_tensor("k1L", [L, GS], FP16, kind="ExternalInput")
    k1x = nc.dram_tensor("k1x", [F, GS], FP16, kind="ExternalInput")
    rk1 = nc.dram_tensor("rk1", [U, GS], FP16, kind="ExternalInput")
    k2 = nc.dram_tensor("k2", [U, GS], FP16, kind="ExternalInput")
    rk2 = nc.dram_tensor("rk2", [U, GS], FP16, kind="ExternalInput")
    k3 = nc.dram_tensor("k3", [U, GS], FP16, kind="ExternalInput")
    rk3 = nc.dram_tensor("rk3", [U, GS], FP16, kind="ExternalInput")
    wp = nc.dram_tensor("wp", [U, V], FP16, kind="ExternalInput")
    b1r = nc.dram_tensor("b1r", [1, GS], FP16, kind="ExternalInput")
    b2r = nc.dram_tensor("b2r", [1, GS], FP16, kind="ExternalInput")
    b3r = nc.dram_tensor("b3r", [1, GS], FP16, kind="ExternalInput")
    bvr = nc.dram_tensor("bvr", [1, V], FP16, kind="ExternalInput")
    eye64 = nc.dram_tensor("eye64", [B, B], FP16, kind="ExternalInput")
    eye128 = nc.dram_tensor("eye128", [128, 128], FP16, kind="ExternalInput")
    ones1 = nc.dram_tensor("ones1", [1, B], FP16, kind="ExternalInput")

    y_out = nc.dram_tensor("y_out", [B, TWIN if TT == T else TT, V], F32,
                           kind="ExternalOutput")
    l_out = nc.dram_tensor("l_out", [B, TWIN if TT == T else TT, V], F32,
                           kind="ExternalOutput")

    ydram = nc.dram_tensor("ydram", [B, TT, V], F32)
    ldram = nc.dram_tensor("ldram", [B, TT, V], F32)
    din1 = [nc.dram_tensor(f"din1{p}", [128, B], FP16) for p in range(2)]
    dout1 = [nc.dram_tensor(f"dout1{p}", [U, B], FP16, addr_space="Shared")
             for p in range(2)]
    din23 = [nc.dram_tensor(f"din23{p}", [128, 2, B], FP16) for p in range(2)]
    dout23 = [nc.dram_tensor(f"dout23{p}", [U, 2, B], FP16,
                             addr_space="Shared") for p in range(2)]
    RG = [list(range(NCORE))]

    XCH = 32  # x chunk steps
    ORD = {0: 0, 1: 2, 2: 1}  # region order within a slot: [h1 | h3 | h2]

    with TileContext(nc) as tc:
        with (
            tc.tile_pool(name="wpool", bufs=1) as wpool,
            tc.tile_pool(name="state", bufs=1) as state,
            tc.tile_pool(name="work", bufs=2) as work,
            tc.tile_pool(name="ps", bufs=2, space="PSUM") as ps,
            tc.tile_pool(name="pslb", bufs=1, space="PSUM") as pslb,
            tc.tile_pool(name="pstr", bufs=2, space="PSUM") as pstr,
        ):
            # ---- resident weights
            def load_w(name, dram, kt, ncol):
                t = wpool.tile([128, kt, ncol], FP16, name=name, tag=name)
                nc.sync.dma_start(
                    t[:, :, :], dram.ap().rearrange("(k p) g -> p k g", p=128))
                return t

            rk1s = load_w("rk1s", rk1, KT, GS)
            rk2s = load_w("rk2s", rk2, KT, GS)
            rk3s = load_w("rk3s", rk3, KT, GS)
            k2s = load_w("k2s", k2, KT, GS)
            k3s = load_w("k3s", k3, KT, GS)
            ws = load_w("ws", wp, KT, V)
            k1Ls = load_w("k1Ls", k1L, L // 128, GS)
            latTs = load_w("latTs", latT, L // 128, B)
            k1xs = wpool.tile([F, GS], FP16, tag="k1xs")
            nc.sync.dma_start(k1xs[:, :], k1x[:, :])
            e64 = wpool.tile([B, B], FP16, tag="e64")
            nc.sync.dma_start(e64[:, :], eye64[:, :])
            e128 = wpool.tile([128, 128], FP16, tag="e128")
            nc.sync.dma_start(e128[:, :], eye128[:, :])
            on1 = wpool.tile([1, B], FP16, tag="on1")
            nc.sync.dma_start(on1[:, :], ones1[:, :])
            b2s = wpool.tile([1, GS], FP16, tag="b2s")
            nc.sync.dma_start(b2s[:, :], b2r[:, :])
            b3s = wpool.tile([1, GS], FP16, tag="b3s")
            nc.sync.dma_start(b3s[:, :], b3r[:, :])
            bvs = wpool.tile([1, V], FP16, tag="bvs")
            nc.sync.dma_start(bvs[:, :], bvr[:, :])
            b1s = wpool.tile([1, GS], FP16, tag="b1s")
            nc.sync.dma_start(b1s[:, :], b1r[:, :])

            # ---- latentb = latT.T @ k1L + b1  (once)
            lb_ps = pslb.tile([B, GS], F32, tag="lbps")
            for k in range(L // 128):
                nc.tensor.matmul(lb_ps[:, :], latTs[:, k, :], k1Ls[:, k, :],
                                 start=(k == 0), stop=False)
            nc.tensor.matmul(lb_ps[:, :], on1[:, :], b1s[:, :],
                             start=False, stop=True)
            latentb = state.tile([B, GS], FP16, tag="latentb")
            nc.vector.tensor_copy(latentb[:, :], lb_ps[:, :])
            psD = pslb.tile([128, GS], F32, name="psD", tag="psD")

            # ---- states
            cAB = state.tile([128, US], F32, name="cAB", tag="cAB")
            nc.vector.memset(cAB[:, :], 0.0)
            c3t = state.tile([B, US], F32, name="c3t", tag="c3t")
            nc.vector.memset(c3t[:, :], 0.0)
            # h1a: gathered h1 [128, k, B]; h23a: gathered {h3, h2}
            # [128, l, k, B] (l=0 h3, l=1 h2). Separate tiles so phase-a
            # matmuls (h1 consumers) never wait on the AG23 DMA.
            h1a = [state.tile([128, KT * B], FP16, name=f"h1a{p}",
                              tag=f"h1a{p}") for p in range(2)]
            h23a = [state.tile([128, 2 * KT * B], FP16, name=f"h23a{p}",
                               tag=f"h23a{p}") for p in range(2)]
            for h in h1a + h23a:
                nc.vector.memset(h[:, :].bitcast(F32), 0.0)
            evac1 = [state.tile([128, B], FP16, name=f"evac1{p}",
                                tag=f"evac1{p}") for p in range(2)]
            evac23 = [state.tile([128, 2 * B], FP16, name=f"evac23{p}",
                                 tag=f"evac23{p}") for p in range(2)]
            for e in evac1 + evac23:
                nc.vector.memset(e[:, :].bitcast(F32), 0.0)

            def h1_slice(hbuf, k):
                return hbuf[:, k * B:(k + 1) * B]

            def h23_slice(hbuf, layer, k):
                # layer 1 -> l=1 (h2), layer 2 -> l=0 (h3)
                l = 0 if layer == 2 else 1
                off = (l * KT + k) * B
                return hbuf[:, off:off + B]

            xsb = [state.tile([F, XCH, B], FP16, name=f"xsb{p}", tag=f"xsb{p}")
                   for p in range(2)]

            nsteps = TT + 4
            for s in range(nsteps):
                par = s % 2
                hp1 = h1a[(s + 1) % 2]    # written by AG1(s-1)
                hp23 = h23a[(s + 1) % 2]  # written by AG23(s-1)
                hp23o = h23a[s % 2]       # written by AG23(s-2) (old)
                t1, t2, t3, tpj = s, s - 1, s - 2, s - 4
                do1 = 0 <= t1 < TT
                do2 = 0 <= t2 < TT
                do3 = 0 <= t3 < TT
                dopj = 0 <= tpj < TT

                psAB = ps.tile([128, GS], F32, name=f"psAB{s}", tag="psAB")
                psB = ps.tile([128, V], F32, name=f"psB{s}", tag="psB")

                if do1 and t1 % XCH == 0:
                    xb = xsb[(t1 // XCH) % 2]
                    hi = min(XCH, TT - t1)
                    nc.sync.dma_start(xb[:, 0:hi, :], xT[:, t1:t1 + hi, :])

                # ---- emission queues. *_a needs only bcast1 round cnt1 /
                # older data; *_b needs bcast23 round cnt23.
                cg0_a, cg0_b = [], []    # psum partitions 0:64
                cg64_a, cg64_b = [], []  # psum partitions 64:128
                cg0_e, cg64_e = [], []   # "early": no AG1(s-1) dependency
                if do1:
                    g1 = psAB[0:B, :]
                    xb = xsb[(t1 // XCH) % 2]
                    cg0_e.append(lambda: nc.tensor.matmul(
                        g1, xb[:, t1 % XCH, :], k1xs[:, :],
                        start=True, stop=False))
                    cg0_e.append(lambda: nc.tensor.matmul(
                        g1, e64[:, :], latentb[:, :], start=False, stop=False))
                    for k in range(KT):
                        cg0_a.append(lambda k=k: nc.tensor.matmul(
                            g1, h1_slice(hp1, k), rk1s[:, k, :],
                            start=False, stop=(k == KT - 1)))
                if dopj:
                    gp = psB[B:128, :]
                    cg64_e.append(lambda: nc.tensor.matmul(
                        gp, on1[:, :], bvs[:, :], start=True, stop=False))
                    for k in range(KT):
                        # h3(s-4): AG23(s-2) -> no fresh dependency
                        cg64_e.append(lambda k=k: nc.tensor.matmul(
                            gp, h23_slice(hp23o, 2, k), ws[:, k, :],
                            start=False, stop=(k == KT - 1)))
                if do2:
                    g2 = psAB[B:128, :]
                    for k in range(KT):
                        # h1(s-1): AG1(s-1)
                        cg64_a.append(lambda k=k: nc.tensor.matmul(
                            g2, h1_slice(hp1, k), k2s[:, k, :],
                            start=(k == 0), stop=False))
                    for k in range(KT):
                        # h2(s-2): bcast23 round cnt23
                        cg64_b.append(lambda k=k: nc.tensor.matmul(
                            g2, h23_slice(hp23, 1, k), rk2s[:, k, :],
                            start=False, stop=False))
                    cg64_b.append(lambda: nc.tensor.matmul(
                        g2, on1[:, :], b2s[:, :], start=False, stop=True))
                if do3:
                    g3 = psB[0:B, :]
                    for k in range(KT):
                        # h3(s-3): bcast23 round cnt23
                        cg0_b.append(lambda k=k: nc.tensor.matmul(
                            g3, h23_slice(hp23, 2, k),
                            rk3s[:, k, :], start=(k == 0), stop=False))
                    cg0_b.append(lambda: nc.tensor.matmul(
                        g3, on1[:, :], b3s[:, :], start=False, stop=False))
                    for k in range(KT):
                        # h2(s-2): round cnt23
                        cg0_b.append(lambda k=k: nc.tensor.matmul(
                            g3, h23_slice(hp23, 1, k), k3s[:, k, :],
                            start=False, stop=(k == KT - 1)))

                gsbA = work.tile([128, GS], F32, name=f"gsbA{s}", tag="gsbA")
                gsb3 = work.tile([B, GS], F32, name=f"gsb3{s}", tag="gsb3")
                hsb1 = work.tile([B, US], FP16, name=f"h1_{s}", tag="h1")
                hsb23 = work.tile([128, US], FP16, name=f"h23_{s}", tag="h23")
                tpC = pstr.tile([128, 192], FP16, name=f"tpC_{s}", tag="tpC")
                tp1 = tpC[:, 0:64]
                tp23 = tpC[:, 64:192]

                tmpAB = work.tile([128, US], F32, name=f"tmpAB{s}",
                                  tag="tmpAB")
                thAB = work.tile([128, US], F32, name=f"thAB{s}", tag="thAB")
                tmp3 = work.tile([B, US], F32, name=f"tmp3_{s}", tag="tmp3")
                th3 = work.tile([B, US], F32, name=f"th3_{s}", tag="th3")

                def cell_vec(gs, c, hout, tmp, th):
                    nc.vector.tensor_tensor(tmp, gs[:, 0:128],
                                            gs[:, 384:512], ALU.mult)
                    nc.vector.tensor_tensor(c, gs[:, 128:256], c, ALU.mult)
                    nc.vector.tensor_tensor(c, c, tmp, ALU.add)
                    nc.scalar.activation(th, c, ACT.Tanh)
                    nc.vector.tensor_tensor(hout, gs[:, 256:384], th,
                                            ALU.mult)

                def l1_act():
                    g1 = psAB[0:B, :]
                    nc.scalar.activation(gsbA[0:B, 0:384], g1[:, 0:384],
                                         ACT.Sigmoid)
                    nc.scalar.activation(gsbA[0:B, 384:512], g1[:, 384:512],
                                         ACT.Tanh)
                    cell_vec(gsbA[0:B, :], cAB[0:B, :], hsb1[:, :],
                             tmpAB[0:B, :], thAB[0:B, :])

                # ---- phase a-early: no dependency on AG1(s-1); runs right
                # after phase-b(s-1), keeping the PE warm while AG1(s-1) +
                # AG23(s-1) are in flight.
                nE = max(len(cg0_e), len(cg64_e))
                for i in range(nE):
                    if i < len(cg0_e):
                        cg0_e[i]()
                    if i < len(cg64_e):
                        cg64_e[i]()
                if 0 < s < TT:
                    for _ in range(4):
                        nc.tensor.matmul(psD[0:B, :], e64[:, :],
                                         latentb[:, :], start=True, stop=True)
                        nc.tensor.matmul(psD[B:128, :], e64[:, :],
                                         latentb[:, :], start=True, stop=True)

                # ---- phase a: rk1 || k2h1 (both gated on AG1(s-1))
                na = len(cg0_a)
                nA = max(na, len(cg64_a))
                for i in range(nA):
                    if i < na:
                        cg0_a[i]()
                    if do1 and i == na - 1:
                        l1_act()
                    if i < len(cg64_a):
                        cg64_a[i]()

                # AG1 chain emitted immediately: its trigger is data-ready
                # mid-AG23(s-1), so the collective core takes it the moment
                # it frees.
                if do1:
                    nc.tensor.transpose(tp1, hsb1[:, :], e64[:, :])
                    nc.vector.tensor_copy(evac1[par][:, :], tp1)
                    nc.sync.dma_start(din1[par].ap(), evac1[par][:, :])
                    nc.gpsimd.collective_compute(
                        "AllGather", ALU.bypass, replica_groups=RG,
                        ins=[din1[par].ap().opt()],
                        outs=[dout1[par].ap().opt()],
                    )
                    nc.sync.dma_start(
                        h1a[par][:, :].rearrange("p (k b) -> p k b", k=KT),
                        dout1[par].ap().rearrange("(k p) b -> p k b", p=128),
                    )

                # ---- phase b (needs AG23(s-1))
                rest64 = cg64_b
                nb = max(len(cg0_b), len(rest64))
                for i in range(nb):
                    if i < len(cg0_b):
                        cg0_b[i]()
                    if i < len(rest64):
                        rest64[i]()

                # ---- L2 / L3 activations
                if do2:
                    g2 = psAB[B:128, :]
                    nc.scalar.activation(gsbA[B:128, 0:384], g2[:, 0:384],
                                         ACT.Sigmoid)
                    nc.scalar.activation(gsbA[B:128, 384:512], g2[:, 384:512],
                                         ACT.Tanh)
                    cell_vec(gsbA[B:128, :], cAB[B:128, :], hsb23[B:128, :],
                             tmpAB[B:128, :], thAB[B:128, :])
                if do3:
                    g3 = psB[0:B, :]
                    nc.scalar.activation(gsb3[:, 0:384], g3[:, 0:384],
                                         ACT.Sigmoid)
                    nc.scalar.activation(gsb3[:, 384:512], g3[:, 384:512],
                                         ACT.Tanh)
                    cell_vec(gsb3[:, :], c3t[:, :], hsb23[0:B, :],
                             tmp3[:, :], th3[:, :])

                # ---- broadcast {h3, h2}
                if do2 or do3:
                    if not do3:
                        nc.vector.memset(hsb23[0:B, :].bitcast(F32), 0.0)
                    if not do2:
                        nc.vector.memset(hsb23[B:128, :].bitcast(F32), 0.0)
                    nc.tensor.transpose(tp23, hsb23[:, :], e128[:, :])
                    nc.vector.tensor_copy(evac23[par][:, :], tp23)
                    nc.sync.dma_start(
                        din23[par].ap().rearrange("p l b -> p (l b)"),
                        evac23[par][:, :])
                    nc.gpsimd.collective_compute(
                        "AllGather", ALU.bypass, replica_groups=RG,
                        ins=[din23[par].ap().opt()],
                        outs=[dout23[par].ap().opt()],
                    )
                    nc.sync.dma_start(
                        h23a[par][:, :].rearrange("p (l k b) -> p l k b",
                                                  l=2, k=KT),
                        dout23[par].ap().rearrange("(k p) l b -> p l k b",
                                                   p=128),
                    )

                # ---- softmax + outputs for step tpj
                if dopj:
                    gp = psB[B:128, :]
                    lsb = work.tile([128, V], F32, name=f"lsb{tpj}", tag="lsb")
                    nc.vector.tensor_copy(lsb[B:128, :], gp)
                    nmx = work.tile([128, 1], F32, name=f"nmx{tpj}", tag="nmx")
                    nc.vector.tensor_reduce(nmx[B:128, :], lsb[B:128, :],
                                            axis=mybir.AxisListType.X,
                                            op=ALU.max, negate=True)
                    esb = work.tile([128, V], F32, name=f"esb{tpj}", tag="esb")
                    nc.scalar.activation(esb[B:128, :], lsb[B:128, :], ACT.Exp,
                                         bias=nmx[B:128, 0:1])
                    sm = work.tile([128, 1], F32, name=f"sm{tpj}", tag="sm")
                    nc.vector.tensor_reduce(sm[B:128, :], esb[B:128, :],
                                            axis=mybir.AxisListType.X,
                                            op=ALU.add)
                    rs = work.tile([128, 1], F32, name=f"rs{tpj}", tag="rs")
                    nc.vector.reciprocal(rs[B:128, :], sm[B:128, :])
                    ysb = work.tile([128, V], F32, name=f"ysb{tpj}", tag="ysb")
                    nc.vector.tensor_scalar(ysb[B:128, :], esb[B:128, :],
                                            rs[B:128, 0:1], None, ALU.mult)
                    nc.sync.dma_start(ydram[:, tpj:tpj + 1, :], ysb[B:128, :])
                    nc.sync.dma_start(ldram[:, tpj:tpj + 1, :], lsb[B:128, :])

            # windowed output copy (per-core time window)
            if TT == T:
                pid = nc.gpsimd.partition_id()
                off = pid * TWIN
                nc.gpsimd.dma_start(y_out[:, :, :],
                                    ydram[:, bass.ds(off, TWIN), :])
                nc.gpsimd.dma_start(l_out[:, :, :],
                                    ldram[:, bass.ds(off, TWIN), :])
            else:
                nc.gpsimd.dma_start(y_out[:, :, :], ydram[:, :, :])
                nc.gpsimd.dma_start(l_out[:, :, :], ldram[:, :, :])

    nc.compile()
    return nc


_built = {}


def _get_nc(t_steps):
    if t_steps not in _built:
        _built[t_steps] = build(t_steps)
    return _built[t_steps]


def _prep_inputs(latent, x, k1, rk1, b1, k2, rk2, b2, k3, rk3, b3, w, b,
                 t_steps):
    f16 = np.float16
    latent = np.asarray(latent, f16)
    x = np.asarray(x, f16)
    k1 = np.asarray(k1, f16)
    rk1 = np.asarray(rk1, f16)
    k2 = np.asarray(k2, f16)
    rk2 = np.asarray(rk2, f16)
    k3 = np.asarray(k3, f16)
    rk3 = np.asarray(rk3, f16)
    w = np.asarray(w, f16)
    in_maps = []
    latT = np.ascontiguousarray(latent.T)                   # [L, B]
    xT = np.ascontiguousarray(np.transpose(x, (2, 1, 0)))   # [F, T, B]
    eye64v = np.eye(B, dtype=f16)
    eye128v = np.eye(128, dtype=f16)
    ones1v = np.ones((1, B), f16)
    for j in range(NCORE):
        u0 = j * US
        cols = np.concatenate([
            np.arange(u0, u0 + US),                  # i
            np.arange(U + u0, U + u0 + US),          # f
            np.arange(3 * U + u0, 3 * U + u0 + US),  # o
            np.arange(2 * U + u0, 2 * U + u0 + US),  # g
        ])
        in_maps.append({
            "latT": latT,
            "xT": np.ascontiguousarray(xT[:, :t_steps, :]),
            "k1L": np.ascontiguousarray(k1[:L, cols]),
            "k1x": np.ascontiguousarray(k1[L:, cols]),
            "rk1": np.ascontiguousarray(rk1[:, cols]),
            "k2": np.ascontiguousarray(k2[:, cols]),
            "rk2": np.ascontiguousarray(rk2[:, cols]),
            "k3": np.ascontiguousarray(k3[:, cols]),
            "rk3": np.ascontiguousarray(rk3[:, cols]),
            "wp": np.ascontiguousarray(np.asarray(w, f16)),
            "b1r": np.ascontiguousarray(np.asarray(b1, f16)[cols][None, :]),
            "b2r": np.ascontiguousarray(np.asarray(b2, f16)[cols][None, :]),
            "b3r": np.ascontiguousarray(np.asarray(b3, f16)[cols][None, :]),
            "bvr": np.ascontiguousarray(np.asarray(b, f16)[None, :]),
            "eye64": eye64v,
            "eye128": eye128v,
            "ones1": ones1v,
        })
    return in_maps


def run(t_steps=T, **inputs):
    nc = _get_nc(t_steps)
    in_maps = _prep_inputs(t_steps=t_steps, **inputs)
    res = run_bass_kernel_spmd(nc, in_maps, core_ids=list(range(NCORE)))
    if t_steps == T:
        y = np.concatenate([res.results[j]["y_out"] for j in range(NCORE)],
                           axis=1)
        lg = np.concatenate([res.results[j]["l_out"] for j in range(NCORE)],
                            axis=1)
    else:
        y = res.results[0]["y_out"]
        lg = res.results[0]["l_out"]
    return y, lg


def kernel(**inputs):
    return run(t_steps=T, **inputs)
